# revision 1
# baseline (speedup 1.0000x reference)
"""EqPBCNN (perturbation-based nonlinearity compensation NN) Trainium2 Bass kernel.

Data-parallel over 8 NeuronCores: batch 65536 -> 8192 per core.

Math (per sample, per polarization p):
  triplet features  F[h,p] = SYM[h] * (A[h,0]+A[h,1]) * x[m_h,p],
                    A[h,p] = x[n_h,p] * conj(x[m_h+n_h,p])
  h1 = CLrelu(F @ W1^T); h2 = CLrelu(h1 @ W2^T); E = h2 @ W3^T
  out = x[center,p] + E * 10^(task0/10)/2

Kernel pipeline (taps-on-partitions, batch on free dim, chunks of 512):
  gather matmuls (PE)  -> pair stacks A,C (350 rows = (h, pol))
  G products (DVE)     -> G = A * conj(C)
  R matmuls (PE)       -> R[o,m,p] = sum_n W1'[p,o,(m,n)] * (G[h,0]+G[h,1])   (100 rows)
  T products (DVE)     -> T = xrep * R   (complex)
  final matmul (PE)    -> h1[p,o] = sum_m T    (8 rows)
  ACT lrelu / W2 / lrelu / W3 -> E (4 rows); exp for P; residual add; store.
"""
import numpy as np

# ---------------- problem constants (hardcoded; must match reference) -------
BATCH = 65536
MT, LH = 41, 20          # filter taps, half window
NM = 2                   # modes / polarizations
H1, H2 = 2, 10
SLOPE = 0.01
NCORES = 8
BCORE = BATCH // NCORES  # 8192
NB = 512                 # samples per chunk
NCHUNK = BCORE // NB     # 16
ROWS = MT * NM           # 82 = tap*2 + mode

_idx = [(m, n) for m in range(-LH, LH + 1) for n in range(-LH, LH + 1)
        if abs(m * n) <= LH and abs(m + n) <= LH and n >= m]
H = len(_idx)            # 175
M_ARR = np.array([t[0] for t in _idx], np.int32)
N_ARR = np.array([t[1] for t in _idx], np.int32)
A_TAP = N_ARR + LH           # source tap for En
C_TAP = M_ARR + N_ARR + LH   # source tap for Emn (conjugated side)
SYM = np.where(M_ARR != N_ARR, 2.0, 1.0).astype(np.float32)
M_VALS = sorted(set(M_ARR.tolist()))     # 25 distinct m values
NMV = len(M_VALS)
M_POS = {m: i for i, m in enumerate(M_VALS)}
NO = H1 * NMV * NM       # 100 rows of R/T space: (o, mi, p)
NSTACK = 2 * H           # 350 rows: (h, pol)
KSPLITS = [(0, 128), (128, 128), (256, NSTACK - 256)]   # psplits of the stacks


def _orow(o, mi, p):
    return (o * NMV + mi) * NM + p


def _hrow(p, o, comp):
    return (p * H1 + o) * 2 + comp


def _h2row(p, q, comp):
    return (p * H2 + q) * 2 + comp


def build_static():
    """Weight-independent constant matrices."""
    # gather selections: stack row r = 2h+p reads XT row 2*tap+p
    SEL = np.zeros((ROWS, 2 * NSTACK), np.float32)   # [82, 700]: cols 0:350 A, 350:700 C
    for h in range(H):
        for p in range(NM):
            r = 2 * h + p
            SEL[2 * A_TAP[h] + p, r] = 1.0
            SEL[2 * C_TAP[h] + p, NSTACK + r] = 1.0
    # xrep: col (o,mi,p) reads tap m
    XREPW = np.zeros((ROWS, NO), np.float32)
    for o in range(H1):
        for mi, mv in enumerate(M_VALS):
            for p in range(NM):
                XREPW[2 * (mv + LH) + p, _orow(o, mi, p)] = 1.0
    # final contraction [100, 16]: cols 0:8 from Tre, 8:16 from Tim
    FINW = np.zeros((NO, 16), np.float32)
    for o in range(H1):
        for mi in range(NMV):
            for p in range(NM):
                FINW[_orow(o, mi, p), _hrow(p, o, 0)] = 1.0
                FINW[_orow(o, mi, p), 8 + _hrow(p, o, 1)] = 1.0
    return {"SEL": SEL, "XREPW": XREPW, "FINW": FINW}


def fold_weights(W1r, W1i, W2r, W2i, W3r, W3i):
    """Runtime weight folding into matmul lhsT constants."""
    Wr = W1r * SYM[None, None, :]   # [p, o, h]
    Wi = W1i * SYM[None, None, :]
    # R lhsT: [350, 400] cols: 0:100 Gr->Rre, 100:200 Gi->Rre, 200:300 Gr->Rim, 300:400 Gi->Rim
    RW = np.zeros((NSTACK, 4 * NO), np.float32)
    for h in range(H):
        mi = M_POS[M_ARR[h]]
        for p in range(NM):          # output pol (weights are per-pol)
            for q in range(NM):      # source stack row pol (pol-swap fold)
                r = 2 * h + q
                for o in range(H1):
                    c = _orow(o, mi, p)
                    RW[r, 0 * NO + c] += Wr[p, o, h]
                    RW[r, 1 * NO + c] -= Wi[p, o, h]
                    RW[r, 2 * NO + c] += Wi[p, o, h]
                    RW[r, 3 * NO + c] += Wr[p, o, h]
    RWP = np.zeros((3, 128, 4 * NO), np.float32)
    for k, (r0, rk) in enumerate(KSPLITS):
        RWP[k, :rk, :] = RW[r0:r0 + rk, :]
    # W2 lhsT [8, 40]
    W2L = np.zeros((8, 2 * H2 * NM), np.float32)
    for p in range(NM):
        for q in range(H2):
            for o in range(H1):
                W2L[_hrow(p, o, 0), _h2row(p, q, 0)] += W2r[p, q, o]
                W2L[_hrow(p, o, 1), _h2row(p, q, 0)] -= W2i[p, q, o]
                W2L[_hrow(p, o, 0), _h2row(p, q, 1)] += W2i[p, q, o]
                W2L[_hrow(p, o, 1), _h2row(p, q, 1)] += W2r[p, q, o]
    # W3 lhsT [40, 4]: out rows (comp, p): [re_p0, re_p1, im_p0, im_p1]; 1/NM folded
    W3L = np.zeros((2 * H2 * NM, 4), np.float32)
    s = 1.0 / NM
    for p in range(NM):
        for q in range(H2):
            W3L[_h2row(p, q, 0), 0 + p] += W3r[p, 0, q] * s
            W3L[_h2row(p, q, 1), 0 + p] -= W3i[p, 0, q] * s
            W3L[_h2row(p, q, 0), 2 + p] += W3i[p, 0, q] * s
            W3L[_h2row(p, q, 1), 2 + p] += W3r[p, 0, q] * s
    return {"RWP": RWP, "W2L": W2L, "W3L": W3L}


# ---------------------------------------------------------------------------
def build_nc(bcore=BCORE, mm_dtype_name="float32r", nb=NB, lrelu_mode="act"):
    """Build the Bass program for one core processing `bcore` samples."""
    import concourse.bass as bass
    import concourse.bacc as bacc
    import concourse.mybir as mybir
    from concourse.tile import TileContext
    import bass_rust

    nchunk = bcore // nb
    assert nchunk * nb == bcore
    grp = 4 if nchunk % 4 == 0 else 1
    f32 = mybir.dt.float32
    mmdt = getattr(mybir.dt, mm_dtype_name)
    AF = bass_rust.ActivationFunctionType
    OP = mybir.AluOpType

    nc = bacc.Bacc(None, target_bir_lowering=False, debug=False)
    xTr = nc.declare_dram_parameter("xTr", [ROWS + 2, bcore], f32, isOutput=False)
    xTi = nc.declare_dram_parameter("xTi", [ROWS, bcore], f32, isOutput=False)
    selD = nc.declare_dram_parameter("SEL", [ROWS, 2 * NSTACK], f32, isOutput=False)
    xrwD = nc.declare_dram_parameter("XREPW", [ROWS, NO], f32, isOutput=False)
    finD = nc.declare_dram_parameter("FINW", [NO, 16], f32, isOutput=False)
    rwD = nc.declare_dram_parameter("RWP", [3, 128, 4 * NO], f32, isOutput=False)
    w2D = nc.declare_dram_parameter("W2L", [8, 40], f32, isOutput=False)
    w3D = nc.declare_dram_parameter("W3L", [40, 4], f32, isOutput=False)
    outs_d = [nc.declare_dram_parameter(f"out{j}", [4, grp * nb], f32, isOutput=True)
              for j in range(max(1, nchunk // grp))]

    use_r = mm_dtype_name != "float32"

    def r_(ap):   # matmul operands are already declared in the matmul dtype
        return ap

    with TileContext(nc) as tc:
        with (
            tc.tile_pool(name="consts", bufs=1) as cp,
            tc.tile_pool(name="xt", bufs=3) as xp,
            tc.tile_pool(name="g", bufs=2) as gp,
            tc.tile_pool(name="tmp", bufs=2) as tp,
            tc.tile_pool(name="tt", bufs=2) as ttp,
            tc.tile_pool(name="small", bufs=3) as sp,
            tc.tile_pool(name="psum", bufs=4, space="PSUM") as pp,
        ):
            def const_tile(src_ap, name):
                t32 = cp.tile(list(src_ap.shape), f32, name=name + "_32")
                nc.gpsimd.dma_start(out=t32[:], in_=src_ap)
                if not use_r:
                    return t32
                tr = cp.tile(list(src_ap.shape), mmdt, name=name)
                nc.vector.tensor_copy(tr[:], t32[:])
                return tr

            sel_sb = const_tile(selD[:], "sel")
            xrw_sb = const_tile(xrwD[:], "xrw")
            fin_sb = const_tile(finD[:], "fin")
            rw_sb = [const_tile(rwD[k], f"rw{k}") for k in range(3)]
            w2_sb = const_tile(w2D[:], "w2")
            w3_sb = const_tile(w3D[:], "w3")

            for c in range(nchunk):
                cs = slice(c * nb, (c + 1) * nb)
                # ---- load transposed x chunk [82, nb]
                # HWDGE f32 loads + ACT bf16 cast: keeps DMA-trigger ucode off the
                # Pool queue, which runs the G/T adds
                xr32 = xp.tile([98, nb], f32, tag="xr32", bufs=3)
                xi32 = xp.tile([ROWS, nb], f32, tag="xi32", bufs=3)
                nc.sync.dma_start(out=xr32[0:ROWS], in_=xTr[0:ROWS, cs])
                nc.sync.dma_start(out=xr32[96:98], in_=xTr[ROWS:ROWS + 2, cs])
                nc.sync.dma_start(out=xi32[:], in_=xTi[:, cs])
                xr = xp.tile([98, nb], mmdt, tag="xr", bufs=3)
                xi = xp.tile([ROWS, nb], mmdt, tag="xi", bufs=3)
                nc.scalar.copy(xr[0:ROWS], xr32[0:ROWS])
                nc.scalar.copy(xr[96:98], xr32[96:98])
                nc.scalar.copy(xi[:], xi32[:])

                # ---- gather matmuls + G products per psplit
                g_tiles = []
                for k, (r0, rk) in enumerate(KSPLITS):
                    pa_r = pp.tile([128, nb], f32, tag="pp")
                    pa_i = pp.tile([128, nb], f32, tag="pp")
                    pc_r = pp.tile([128, nb], f32, tag="pp")
                    pc_i = pp.tile([128, nb], f32, tag="pp")
                    a_sl = sel_sb[:, r0:r0 + rk]
                    c_sl = sel_sb[:, NSTACK + r0:NSTACK + r0 + rk]
                    # C-side first, copy each to SBUF right after its matmul so
                    # ACT feeds the DVE products with minimal latency
                    cr_s = tp.tile([128, nb], f32, tag="crs", bufs=4)
                    ci_s = tp.tile([128, nb], f32, tag="cis", bufs=4)
                    nc.tensor.matmul(pc_r[:rk], r_(c_sl), r_(xr[:ROWS]), start=True, stop=True)
                    nc.scalar.copy(cr_s[:rk], pc_r[:rk])
                    nc.tensor.matmul(pc_i[:rk], r_(c_sl), r_(xi[:]), start=True, stop=True)
                    nc.scalar.copy(ci_s[:rk], pc_i[:rk])
                    nc.tensor.matmul(pa_r[:rk], r_(a_sl), r_(xr[:ROWS]), start=True, stop=True)
                    nc.tensor.matmul(pa_i[:rk], r_(a_sl), r_(xi[:]), start=True, stop=True)
                    # G = A * conj(C)
                    t0 = tp.tile([128, nb], f32, tag="t0", bufs=4)
                    t1 = tp.tile([128, nb], f32, tag="t1", bufs=4)
                    gr = gp.tile([128, nb], mmdt, tag=f"gr{k}")
                    gi = gp.tile([128, nb], mmdt, tag=f"gi{k}")
                    nc.vector.tensor_tensor(t0[:rk], pa_r[:rk], cr_s[:rk], op=OP.mult)
                    nc.vector.tensor_tensor(t1[:rk], pa_i[:rk], ci_s[:rk], op=OP.mult)
                    nc.gpsimd.tensor_tensor(gr[:rk], t0[:rk], t1[:rk], op=OP.add)
                    nc.vector.tensor_tensor(t0[:rk], pa_i[:rk], cr_s[:rk], op=OP.mult)
                    nc.vector.tensor_tensor(t1[:rk], pa_r[:rk], ci_s[:rk], op=OP.mult)
                    nc.gpsimd.tensor_tensor(gi[:rk], t0[:rk], t1[:rk], op=OP.subtract)
                    g_tiles.append((gr, gi))

                # ---- R matmuls: accumulate over 3 psplits x (Gr, Gi)
                p_rre = pp.tile([128, nb], f32, tag="racc", bufs=2)
                p_rim = pp.tile([128, nb], f32, tag="racc", bufs=2)
                for k, (r0, rk) in enumerate(KSPLITS):
                    gr, gi = g_tiles[k]
                    rw = rw_sb[k]
                    nc.tensor.matmul(p_rre[:NO], r_(rw[:rk, 0:NO]), r_(gr[:rk]),
                                     start=(k == 0), stop=False)
                    nc.tensor.matmul(p_rre[:NO], r_(rw[:rk, NO:2 * NO]), r_(gi[:rk]),
                                     start=False, stop=(k == 2))
                    nc.tensor.matmul(p_rim[:NO], r_(rw[:rk, 2 * NO:3 * NO]), r_(gr[:rk]),
                                     start=(k == 0), stop=False)
                    nc.tensor.matmul(p_rim[:NO], r_(rw[:rk, 3 * NO:4 * NO]), r_(gi[:rk]),
                                     start=False, stop=(k == 2))

                # ---- xrep + center matmuls
                p_xr = pp.tile([128, nb], f32, tag="misc", bufs=2)
                p_xi = pp.tile([128, nb], f32, tag="misc", bufs=2)
                nc.tensor.matmul(p_xr[:NO], r_(xrw_sb[:]), r_(xr[:ROWS]), start=True, stop=True)
                nc.tensor.matmul(p_xi[:NO], r_(xrw_sb[:]), r_(xi[:]), start=True, stop=True)

                # ---- T products (complex xrep * R); R copied to SBUF first
                rre_s = tp.tile([NO, nb], f32, tag="rres")
                rim_s = tp.tile([NO, nb], f32, tag="rims")
                nc.scalar.copy(rre_s[:], p_rre[:NO])
                nc.scalar.copy(rim_s[:], p_rim[:NO])
                u0 = tp.tile([128, nb], f32, tag="u0")
                u1 = tp.tile([128, nb], f32, tag="u1")
                t_re = ttp.tile([NO, nb], mmdt, tag="tre")
                t_im = ttp.tile([NO, nb], mmdt, tag="tim")
                nc.vector.tensor_tensor(u0[:NO], p_xr[:NO], rre_s[:], op=OP.mult)
                nc.vector.tensor_tensor(u1[:NO], p_xi[:NO], rim_s[:], op=OP.mult)
                nc.gpsimd.tensor_tensor(t_re[:], u0[:NO], u1[:NO], op=OP.subtract)
                nc.vector.tensor_tensor(u0[:NO], p_xr[:NO], rim_s[:], op=OP.mult)
                nc.vector.tensor_tensor(u1[:NO], p_xi[:NO], rre_s[:], op=OP.mult)
                nc.gpsimd.tensor_tensor(t_im[:], u0[:NO], u1[:NO], op=OP.add)

                # ---- final contraction -> h1 [8, nb]
                p_h1 = pp.tile([128, nb], f32, tag="misc", bufs=2)
                nc.tensor.matmul(p_h1[:8], r_(fin_sb[:, 0:8]), r_(t_re[:]), start=True, stop=False)
                nc.tensor.matmul(p_h1[:8], r_(fin_sb[:, 8:16]), r_(t_im[:]), start=False, stop=True)

                # ---- MLP tail
                def lrelu(dst, src, rows):
                    if lrelu_mode == "act":
                        nc.scalar.activation(dst[:rows], src[:rows], AF.Lrelu, alpha=SLOPE)
                    else:
                        nc.vector.tensor_scalar_mul(dst[:rows], src[:rows], SLOPE)
                        nc.vector.tensor_tensor(dst[:rows], dst[:rows], src[:rows], op=OP.max)

                h1s = sp.tile([8, nb], mmdt, tag="h1s")
                lrelu(h1s, p_h1, 8)
                p_h2 = pp.tile([128, nb], f32, tag="misc", bufs=2)
                nc.tensor.matmul(p_h2[:40], r_(w2_sb[:]), r_(h1s[:8]), start=True, stop=True)
                h2s = sp.tile([40, nb], mmdt, tag="h2s")
                lrelu(h2s, p_h2, 40)
                p_ere = pp.tile([128, nb], f32, tag="misc", bufs=2)
                p_eim = pp.tile([128, nb], f32, tag="misc", bufs=2)
                nc.tensor.matmul(p_ere[:2], r_(w3_sb[:, 0:2]), r_(h2s[:]), start=True, stop=True)
                nc.tensor.matmul(p_eim[:2], r_(w3_sb[:, 2:4]), r_(h2s[:]), start=True, stop=True)

                # ---- P = 10^(t/10) = exp(t * ln10/10); out = center + E*P
                pex = sp.tile([2, nb], f32, tag="pex")
                nc.scalar.activation(pex[:], xr[96:98], AF.Exp,
                                     scale=float(np.log(10.0) / 10.0))
                if c % grp == 0:
                    cs4 = slice(c * nb, (c + grp) * nb)
                    o_re2 = sp.tile([2, grp * nb], f32, tag="ore", bufs=max(1, nchunk // grp), name="o_re2")
                    o_im2 = sp.tile([2, grp * nb], f32, tag="oim", bufs=max(1, nchunk // grp), name="o_im2")
                    nc.sync.dma_start(out=o_re2[:], in_=xTr[2 * LH:2 * LH + 2, cs4])
                    nc.sync.dma_start(out=o_im2[:], in_=xTi[2 * LH:2 * LH + 2, cs4])
                    chunk_pair = (o_re2, o_im2)
                half = (c % grp) * nb
                o_re = chunk_pair[0][:, half:half + nb]
                o_im = chunk_pair[1][:, half:half + nb]
                ep_r = tp.tile([2, nb], f32, tag="epr")
                ep_i = tp.tile([2, nb], f32, tag="epi")
                nc.vector.tensor_tensor(ep_r[:], p_ere[:2], pex[:], op=OP.mult)
                nc.vector.tensor_tensor(ep_i[:], p_eim[:2], pex[:], op=OP.mult)
                nc.gpsimd.tensor_tensor(o_re, ep_r[:], o_re, op=OP.add)
                nc.gpsimd.tensor_tensor(o_im, ep_i[:], o_im, op=OP.add)
                if c % grp == grp - 1:
                    od = outs_d[c // grp]
                    nc.sync.dma_start(out=od[0:2, :], in_=chunk_pair[0][:])
                    nc.sync.dma_start(out=od[2:4, :], in_=chunk_pair[1][:])
    nc.compile()
    return nc


def _prep_core_inputs(inputs, static, folded):
    """Shard + lay out inputs per core. Returns list of in_maps."""
    xr = np.ascontiguousarray(inputs["x_real"]).reshape(BATCH, ROWS)
    xi = np.ascontiguousarray(inputs["x_imag"]).reshape(BATCH, ROWS)
    t0 = np.ascontiguousarray(inputs["task_info"][:, 0])
    shared = {
        "SEL": static["SEL"], "XREPW": static["XREPW"],
        "FINW": static["FINW"], "RWP": folded["RWP"], "W2L": folded["W2L"],
        "W3L": folded["W3L"],
    }
    in_maps = []
    for c in range(NCORES):
        s = slice(c * BCORE, (c + 1) * BCORE)
        m = dict(shared)
        m["xTr"] = np.ascontiguousarray(
            np.concatenate([xr[s].T, np.broadcast_to(t0[s][None, :], (2, BCORE))], axis=0))
        m["xTi"] = np.ascontiguousarray(xi[s].T)
        in_maps.append(m)
    return in_maps


_CACHE = {}
KERNEL_MM_DTYPE = "bfloat16"   # matmul operand dtype: bfloat16 | float32r | float32


def kernel(**inputs):
    from concourse.bass_utils import run_bass_kernel_spmd

    static = build_static()
    folded = fold_weights(
        np.asarray(inputs["W1_real"]), np.asarray(inputs["W1_imag"]),
        np.asarray(inputs["W2_real"]), np.asarray(inputs["W2_imag"]),
        np.asarray(inputs["W3_real"]), np.asarray(inputs["W3_imag"]),
    )
    if "nc" not in _CACHE:
        _CACHE["nc"] = build_nc(mm_dtype_name=KERNEL_MM_DTYPE)
    nc = _CACHE["nc"]
    in_maps = _prep_core_inputs(inputs, static, folded)
    res = run_bass_kernel_spmd(nc, in_maps, list(range(NCORES)))
    nseg = NCHUNK // 4
    out = np.empty((BATCH, NM, 2), np.float32)
    for c in range(NCORES):
        o4 = np.concatenate([res.results[c][f"out{j}"] for j in range(nseg)], axis=1)
        s = slice(c * BCORE, (c + 1) * BCORE)
        out[s, 0, 0] = o4[0]
        out[s, 1, 0] = o4[1]
        out[s, 0, 1] = o4[2]
        out[s, 1, 1] = o4[3]
    return out



# revision 10
# speedup vs baseline: 14.4763x; 14.4763x over previous
"""EqPBCNN (perturbation-based nonlinearity compensation NN) Trainium2 Bass kernel.

Data-parallel over 8 NeuronCores: batch 65536 -> 8192 per core.

Math (per sample, per polarization p):
  S_h = sum_p' x[n_h, p'] * conj(x[m_h+n_h, p'])        (pol-summed pair product)
  R[p,o,m] = sum_{n:(m,n)} W1'[p,o,(m,n)] * S_(m,n)
  h1[p,o] = sum_m x[m,p] * R[p,o,m]
  h1 -> CLrelu -> W2 -> CLrelu -> W3 -> E;  out = x[center,p] + E * 10^(task0/10)/2

Structural reductions vs the direct formulation:
  * the 175 (m,n) triplets collapse to 148 distinct unordered tap pairs:
    127 complex off-diagonal pairs (2 x 128-row PE splits) and 21 purely-real
    diagonal pairs |x_t|^2 computed from contiguous x rows (no gather).
  * conjugate-mirror duplicates fold into the R weights as sign flips.
  * the m-value set is contiguous (-20..4): the x_m operand is x[0:50]
    replicated, shipped pre-replicated from HBM.
  * re/im are packed side by side on the free dim, so each gather matmul
    pair lands in ONE combined PSUM tile drained by ONE copy (PE cannot
    read PSUM; only ACT/DVE can, so drain count is the scarce resource).
  * P = 10^(task/10) is precomputed on host and folded through the MLP
    via one multiply on h1 (leaky_relu is positively homogeneous).
  * all products/adds run on bf16 SBUF tiles (Pool cannot access PSUM).
"""
import numpy as np

# ---------------- problem constants (hardcoded; must match reference) -------
BATCH = 65536
MT, LH = 41, 20          # filter taps, half window
NM = 2                   # modes / polarizations
H1, H2 = 2, 10
SLOPE = 0.01
NCORES = 8
BCORE = BATCH // NCORES  # 8192
NB = 512                 # samples per chunk
NCHUNK = BCORE // NB     # 16
ROWS = MT * NM           # 82 = tap*2 + mode

_idx = [(m, n) for m in range(-LH, LH + 1) for n in range(-LH, LH + 1)
        if abs(m * n) <= LH and abs(m + n) <= LH and n >= m]
H = len(_idx)            # 175
M_ARR = np.array([t[0] for t in _idx], np.int32)
N_ARR = np.array([t[1] for t in _idx], np.int32)
SYM = np.where(M_ARR != N_ARR, 2.0, 1.0).astype(np.float32)
M_VALS = list(range(-20, 5))             # the 25 distinct m values (contiguous)
NMV = len(M_VALS)
NO = H1 * NMV * NM       # 100 rows of R/T space: (o, mi, p) o-major

# canonical unordered pair set
_canon = {}
PAIR_INFO = []           # per h: ('d', n) diagonal | ('c', k, conj_flag)
for _h, (_m, _n) in enumerate(_idx):
    _a, _b = _n + LH, _m + _n + LH
    if _a == _b:
        PAIR_INFO.append(('d', _a - LH))
    else:
        _key = (min(_a, _b), max(_a, _b))
        if _key not in _canon:
            _canon[_key] = len(_canon)
        PAIR_INFO.append(('c', _canon[_key], _a > _b))
NOFF = len(_canon)                       # 127 off-diagonal pairs
A_K = np.array([k[0] for k in _canon], np.int32)   # smaller tap
B_K = np.array([k[1] for k in _canon], np.int32)   # larger tap (conjugated)
NSTACK = 2 * NOFF                        # 254 rows (pair, pol)
KSPLITS = [(0, 128), (128, NSTACK - 128)]          # 128 + 126
NDIAG = ROWS                             # diag power rows: full x tile; rows 40..81 weighted


def _orow(o, mi, p):
    return o * (NMV * NM) + 2 * mi + p   # o-major: rows 0:50 o=0, 50:100 o=1


def _hrow(p, o, comp):
    return (p * H1 + o) * 2 + comp


def _h2row(p, q, comp):
    return (p * H2 + q) * 2 + comp


def build_static():
    """Weight-independent constant matrices."""
    # gather selections: cols 0:254 A-side (tap a_k), 254:508 C-side (tap b_k)
    SEL = np.zeros((ROWS, 2 * NSTACK), np.float32)
    for k in range(NOFF):
        for p in range(NM):
            SEL[2 * A_K[k] + p, 2 * k + p] = 1.0
            SEL[2 * B_K[k] + p, NSTACK + 2 * k + p] = 1.0
    # final contraction [100, 32]: h1re = sum(u1) - sum(u2); h1im = sum(u3) + sum(u4)
    FINW = np.zeros((NO, 32), np.float32)
    for o in range(H1):
        for mi in range(NMV):
            for p in range(NM):
                FINW[_orow(o, mi, p), 0 + _hrow(p, o, 0)] = 1.0
                FINW[_orow(o, mi, p), 8 + _hrow(p, o, 0)] = -1.0
                FINW[_orow(o, mi, p), 16 + _hrow(p, o, 1)] = 1.0
                FINW[_orow(o, mi, p), 24 + _hrow(p, o, 1)] = 1.0
    return {"SEL": SEL, "FINW": FINW}


def fold_weights(W1r, W1i, W2r, W2i, W3r, W3i):
    """Runtime weight folding into matmul lhsT constants."""
    Wr = W1r * SYM[None, None, :]   # [p, o, h]
    Wi = W1i * SYM[None, None, :]
    # R lhsT for off-diag G stacks: per split s, [128, 400]
    #   cols 0:100 Gre->Rre, 100:200 Gim->Rre, 200:300 Gre->Rim, 300:400 Gim->Rim
    RW = np.zeros((NSTACK, 4 * NO), np.float32)
    # diag-power lhsT [82, 200]: 0:100 pw->Rre, 100:200 pw->Rim (rows 40..81 live)
    RWD = np.zeros((NDIAG, 2 * NO), np.float32)
    for h in range(H):
        mi = M_VALS.index(M_ARR[h])
        info = PAIR_INFO[h]
        for p in range(NM):
            for o in range(H1):
                c = _orow(o, mi, p)
                wre, wim = Wr[p, o, h], Wi[p, o, h]
                if info[0] == 'd':
                    n = info[1]
                    for q in range(NM):
                        RWD[40 + 2 * n + q, c] += wre
                        RWD[40 + 2 * n + q, NO + c] += wim
                else:
                    _, k, flag = info
                    sgn = -1.0 if flag else 1.0   # S = conj(q) when flag
                    for q in range(NM):
                        r = 2 * k + q
                        RW[r, 0 * NO + c] += wre
                        RW[r, 1 * NO + c] -= wim * sgn
                        RW[r, 2 * NO + c] += wim
                        RW[r, 3 * NO + c] += wre * sgn
    RWP = np.zeros((2, 128, 4 * NO), np.float32)
    for s, (r0, rk) in enumerate(KSPLITS):
        RWP[s, :rk, :] = RW[r0:r0 + rk, :]
    # W2 lhsT [8, 40]
    W2L = np.zeros((8, 2 * H2 * NM), np.float32)
    for p in range(NM):
        for q in range(H2):
            for o in range(H1):
                W2L[_hrow(p, o, 0), _h2row(p, q, 0)] += W2r[p, q, o]
                W2L[_hrow(p, o, 1), _h2row(p, q, 0)] -= W2i[p, q, o]
                W2L[_hrow(p, o, 0), _h2row(p, q, 1)] += W2i[p, q, o]
                W2L[_hrow(p, o, 1), _h2row(p, q, 1)] += W2r[p, q, o]
    # W3 lhsT [40, 4]: cols [re_p0, re_p1, im_p0, im_p1]; 1/NM folded
    W3L = np.zeros((2 * H2 * NM, 4), np.float32)
    s3 = 1.0 / NM
    for p in range(NM):
        for q in range(H2):
            W3L[_h2row(p, q, 0), 0 + p] += W3r[p, 0, q] * s3
            W3L[_h2row(p, q, 1), 0 + p] -= W3i[p, 0, q] * s3
            W3L[_h2row(p, q, 0), 2 + p] += W3i[p, 0, q] * s3
            W3L[_h2row(p, q, 1), 2 + p] += W3r[p, 0, q] * s3
    return {"RWP": RWP, "RWD": RWD, "W2L": W2L, "W3L": W3L}


# ---------------------------------------------------------------------------
def build_nc(bcore=BCORE, nb=NB, lrelu_mode="act"):
    """Build the Bass program for one core processing `bcore` samples."""
    import concourse.bass as bass
    import concourse.bacc as bacc
    import concourse.mybir as mybir
    from concourse.tile import TileContext
    import bass_rust

    nchunk = bcore // nb
    assert nchunk * nb == bcore
    grp = 4 if nchunk % 4 == 0 else 1
    f32 = mybir.dt.float32
    bf16 = mybir.dt.bfloat16
    AF = bass_rust.ActivationFunctionType
    OP = mybir.AluOpType

    nc = bacc.Bacc(None, target_bir_lowering=False, debug=False)
    # x with re/im side by side on the free dim: [82, 2, bcore]
    xD = nc.declare_dram_parameter("xri", [ROWS, 2, bcore], bf16, isOutput=False)
    xmD = nc.declare_dram_parameter("xmri", [NO, 2, bcore], bf16, isOutput=False)
    pexD = nc.declare_dram_parameter("pex8", [8, bcore], bf16, isOutput=False)
    extD = nc.declare_dram_parameter("ext", [4, bcore], f32, isOutput=False)
    selD = nc.declare_dram_parameter("SEL", [ROWS, 2 * NSTACK], f32, isOutput=False)
    finD = nc.declare_dram_parameter("FINW", [NO, 32], f32, isOutput=False)
    rwD = nc.declare_dram_parameter("RWP", [2, 128, 4 * NO], f32, isOutput=False)
    rwdD = nc.declare_dram_parameter("RWD", [NDIAG, 2 * NO], f32, isOutput=False)
    w2D = nc.declare_dram_parameter("W2L", [8, 40], f32, isOutput=False)
    w3D = nc.declare_dram_parameter("W3L", [40, 4], f32, isOutput=False)
    outs_d = [nc.declare_dram_parameter(f"out{j}", [4, grp * nb], f32, isOutput=True)
              for j in range(max(1, nchunk // grp))]

    with TileContext(nc) as tc:
        with (
            tc.tile_pool(name="consts", bufs=1) as cp,
            tc.tile_pool(name="xt", bufs=3) as xp,
            tc.tile_pool(name="g", bufs=2) as gp,
            tc.tile_pool(name="tmp", bufs=2) as tp,
            tc.tile_pool(name="small", bufs=3) as sp,
            tc.tile_pool(name="psum", bufs=1, space="PSUM") as pp,
        ):
            def const_tile(src_ap, name):
                t32 = cp.tile(list(src_ap.shape), f32, name=name + "_32")
                nc.gpsimd.dma_start(out=t32[:], in_=src_ap)
                tr = cp.tile(list(src_ap.shape), bf16, name=name)
                nc.vector.tensor_copy(tr[:], t32[:])
                return tr

            sel_sb = const_tile(selD[:], "sel")
            fin_sb = const_tile(finD[:], "fin")
            rw_sb = [const_tile(rwD[s], f"rw{s}") for s in range(2)]
            rwd_sb = const_tile(rwdD[:], "rwd")
            w2_sb = const_tile(w2D[:], "w2")
            w3_sb = const_tile(w3D[:], "w3")

            # ---------------- software-pipelined chunk stages ----------------
            # PE issue order per iteration i: gathers(i), R(i-1), fin/w2/w3(i-2)
            # so PE never waits on the drain->product chain or the tail chain.
            def lrelu(dst, src, rows):
                if lrelu_mode == "act":
                    nc.scalar.activation(dst[:rows], src[:rows], AF.Lrelu, alpha=SLOPE)
                else:
                    nc.vector.tensor_scalar_mul(dst[:rows], src[:rows], SLOPE)
                    nc.vector.tensor_tensor(dst[:rows], dst[:rows], src[:rows], op=OP.max)

            grp_tiles = {}
            st = {}     # per-chunk live tiles

            def stage_load(c):
                cs = slice(c * nb, (c + 1) * nb)
                xt = xp.tile([ROWS, 2, nb], bf16, tag="xt", bufs=3)
                xm = xp.tile([NO, 2, nb], bf16, tag="xm", bufs=3)
                px8 = xp.tile([8, nb], bf16, tag="px8", bufs=3)
                nc.sync.dma_start(out=xt[:], in_=xD[:, :, cs])
                nc.sync.dma_start(out=xm[:], in_=xmD[:, :, cs])
                nc.sync.dma_start(out=px8[:], in_=pexD[:, cs])
                if c % grp == 0:
                    cs4 = slice(c * nb, (c + grp) * nb)
                    o_re2 = sp.tile([2, grp * nb], f32, tag="ore", bufs=max(1, nchunk // grp), name="o_re2")
                    o_im2 = sp.tile([2, grp * nb], f32, tag="oim", bufs=max(1, nchunk // grp), name="o_im2")
                    nc.sync.dma_start(out=o_re2[:], in_=extD[0:2, cs4])
                    nc.sync.dma_start(out=o_im2[:], in_=extD[2:4, cs4])
                    grp_tiles[c // grp] = (o_re2, o_im2)
                st[c] = {"xt": xt, "xm": xm, "px8": px8}

            def stage_gather(c):
                xt = st[c]["xt"]
                pas, pcs = [], []
                for s, (r0, rk) in enumerate(KSPLITS):
                    pa = pp.tile([128, 2, nb], f32, tag="pa", bufs=1)
                    pc = pp.tile([128, 2, nb], f32, tag="pc", bufs=1)
                    a_sl = sel_sb[:, r0:r0 + rk]
                    c_sl = sel_sb[:, NSTACK + r0:NSTACK + r0 + rk]
                    nc.tensor.matmul(pa[:rk, 0, :], a_sl, xt[:, 0, :], start=True, stop=True)
                    nc.tensor.matmul(pa[:rk, 1, :], a_sl, xt[:, 1, :], start=True, stop=True)
                    nc.tensor.matmul(pc[:rk, 0, :], c_sl, xt[:, 0, :], start=True, stop=True)
                    nc.tensor.matmul(pc[:rk, 1, :], c_sl, xt[:, 1, :], start=True, stop=True)
                    pas.append(pa)
                    pcs.append(pc)
                st[c]["pa"] = pas
                st[c]["pc"] = pcs

            def stage_drain(c):
                dr = []
                for s, (r0, rk) in enumerate(KSPLITS):
                    pa, pc = st[c]["pa"][s], st[c]["pc"][s]
                    aS = tp.tile([128, 2, nb], bf16, tag="aS", bufs=2)
                    cS = tp.tile([128, 2, nb], bf16, tag="cS", bufs=2)
                    if s == 0:
                        nc.scalar.copy(aS[:rk], pa[:rk])
                    else:
                        nc.vector.tensor_copy(aS[:rk], pa[:rk])
                    nc.scalar.copy(cS[:rk], pc[:rk])
                    dr.append((aS, cS))
                st[c]["dr"] = dr

            def stage_products(c):
                xt = st[c]["xt"]
                # diagonal power rows: pw = xre^2 + xim^2 (rows 40..81 weighted)
                ud = sp.tile([NDIAG, 2, nb], bf16, tag="ud", bufs=2)
                pw = gp.tile([NDIAG, nb], bf16, tag="pw", bufs=2)
                nc.gpsimd.tensor_tensor(ud[:], xt[:], xt[:], op=OP.mult)
                nc.vector.tensor_tensor(pw[:], ud[:, 0, :], ud[:, 1, :], op=OP.add)
                g_tiles = []
                for s, (r0, rk) in enumerate(KSPLITS):
                    aS, cS = st[c]["dr"][s]
                    m1 = tp.tile([128, nb], bf16, tag="m1", bufs=2)
                    m2 = tp.tile([128, nb], bf16, tag="m2", bufs=2)
                    m3 = tp.tile([128, nb], bf16, tag="m3", bufs=2)
                    m4 = tp.tile([128, nb], bf16, tag="m4", bufs=2)
                    gr = gp.tile([128, nb], bf16, tag=f"gr{s}")
                    gi = gp.tile([128, nb], bf16, tag=f"gi{s}")
                    nc.gpsimd.tensor_tensor(m1[:rk], aS[:rk, 0, :], cS[:rk, 0, :], op=OP.mult)
                    nc.gpsimd.tensor_tensor(m2[:rk], aS[:rk, 1, :], cS[:rk, 1, :], op=OP.mult)
                    nc.gpsimd.tensor_tensor(m3[:rk], aS[:rk, 1, :], cS[:rk, 0, :], op=OP.mult)
                    nc.gpsimd.tensor_tensor(m4[:rk], aS[:rk, 0, :], cS[:rk, 1, :], op=OP.mult)
                    nc.vector.tensor_tensor(gr[:rk], m1[:rk], m2[:rk], op=OP.add)
                    nc.vector.tensor_tensor(gi[:rk], m3[:rk], m4[:rk], op=OP.subtract)
                    g_tiles.append((gr, gi))
                st[c]["g"] = g_tiles
                st[c]["pw"] = pw

            def stage_R(c):
                g_tiles, pw, xm = st[c]["g"], st[c]["pw"], st[c]["xm"]
                p_R = pp.tile([128, 2, nb], f32, tag="pR", bufs=1)
                for s, (r0, rk) in enumerate(KSPLITS):
                    gr, gi = g_tiles[s]
                    rw = rw_sb[s]
                    nc.tensor.matmul(p_R[:NO, 0, :], rw[:rk, 0:NO], gr[:rk],
                                     start=(s == 0), stop=False)
                    nc.tensor.matmul(p_R[:NO, 0, :], rw[:rk, NO:2 * NO], gi[:rk],
                                     start=False, stop=False)
                    nc.tensor.matmul(p_R[:NO, 1, :], rw[:rk, 2 * NO:3 * NO], gr[:rk],
                                     start=(s == 0), stop=False)
                    nc.tensor.matmul(p_R[:NO, 1, :], rw[:rk, 3 * NO:4 * NO], gi[:rk],
                                     start=False, stop=False)
                nc.tensor.matmul(p_R[:NO, 0, :], rwd_sb[:, 0:NO], pw[:], start=False, stop=True)
                nc.tensor.matmul(p_R[:NO, 1, :], rwd_sb[:, NO:2 * NO], pw[:], start=False, stop=True)
                rS = tp.tile([NO, 2, nb], bf16, tag="rS", bufs=2)
                nc.scalar.copy(rS[:], p_R[:NO])
                # T products (xrep * R), contracted by fin with +- weights next iter
                u1 = tp.tile([NO, nb], bf16, tag="u1", bufs=2)
                u2 = tp.tile([NO, nb], bf16, tag="u2", bufs=2)
                u3 = tp.tile([NO, nb], bf16, tag="u3", bufs=2)
                u4 = tp.tile([NO, nb], bf16, tag="u4", bufs=2)
                nc.gpsimd.tensor_tensor(u1[:], xm[:, 0, :], rS[:, 0, :], op=OP.mult)
                nc.gpsimd.tensor_tensor(u2[:], xm[:, 1, :], rS[:, 1, :], op=OP.mult)
                nc.vector.tensor_tensor(u3[:], xm[:, 0, :], rS[:, 1, :], op=OP.mult)
                nc.vector.tensor_tensor(u4[:], xm[:, 1, :], rS[:, 0, :], op=OP.mult)
                st[c]["u"] = (u1, u2, u3, u4)

            def stage_tail(c):
                u1, u2, u3, u4 = st[c]["u"]
                px8 = st[c]["px8"]
                p_h1 = pp.tile([128, nb], f32, tag="ph1", bufs=1)
                nc.tensor.matmul(p_h1[:8], fin_sb[:, 0:8], u1[:], start=True, stop=False)
                nc.tensor.matmul(p_h1[:8], fin_sb[:, 8:16], u2[:], start=False, stop=False)
                nc.tensor.matmul(p_h1[:8], fin_sb[:, 16:24], u3[:], start=False, stop=False)
                nc.tensor.matmul(p_h1[:8], fin_sb[:, 24:32], u4[:], start=False, stop=True)
                h1s = sp.tile([8, nb], bf16, tag="h1s")
                lrelu(h1s, p_h1, 8)
                # P > 0 and lrelu is positively homogeneous: fold P into h1
                h1p = sp.tile([8, nb], bf16, tag="h1p")
                nc.vector.tensor_tensor(h1p[:], h1s[:], px8[:], op=OP.mult)
                p_h2 = pp.tile([128, nb], f32, tag="misc", bufs=1)
                nc.tensor.matmul(p_h2[:40], w2_sb[:], h1p[:8], start=True, stop=True)
                h2s = sp.tile([40, nb], bf16, tag="h2s")
                lrelu(h2s, p_h2, 40)
                p_ere = pp.tile([128, nb], f32, tag="misc", bufs=1)
                p_eim = pp.tile([128, nb], f32, tag="misc", bufs=1)
                nc.tensor.matmul(p_ere[:2], w3_sb[:, 0:2], h2s[:], start=True, stop=True)
                nc.tensor.matmul(p_eim[:2], w3_sb[:, 2:4], h2s[:], start=True, stop=True)
                # out = center + E' (P already folded into E')
                half = (c % grp) * nb
                o_re = grp_tiles[c // grp][0][:, half:half + nb]
                o_im = grp_tiles[c // grp][1][:, half:half + nb]
                nc.vector.tensor_tensor(o_re, p_ere[:2], o_re, op=OP.add)
                nc.vector.tensor_tensor(o_im, p_eim[:2], o_im, op=OP.add)
                if c % grp == grp - 1:
                    od = outs_d[c // grp]
                    nc.sync.dma_start(out=od[0:2, :], in_=grp_tiles[c // grp][0][:])
                    nc.sync.dma_start(out=od[2:4, :], in_=grp_tiles[c // grp][1][:])
                del st[c]

            for i in range(nchunk + 2):
                if i < nchunk:
                    stage_load(i)
                    stage_gather(i)          # PE
                if i >= 1 and i - 1 < nchunk:
                    stage_R(i - 1)           # PE + rS/u
                if i >= 2:
                    stage_tail(i - 2)        # PE fin + MLP tail
                if i < nchunk:
                    stage_drain(i)           # ACT/DVE
                    stage_products(i)        # POOL/DVE
    nc.compile()
    return nc


def _prep_core_inputs(inputs, static, folded):
    """Shard + lay out inputs per core. Returns list of in_maps."""
    import ml_dtypes
    bf16 = ml_dtypes.bfloat16
    xr = np.ascontiguousarray(inputs["x_real"]).reshape(BATCH, ROWS)
    xi = np.ascontiguousarray(inputs["x_imag"]).reshape(BATCH, ROWS)
    t0 = np.ascontiguousarray(inputs["task_info"][:, 0])
    P = (10.0 ** (t0.astype(np.float64) / 10.0)).astype(np.float32)
    shared = {
        "SEL": static["SEL"], "FINW": static["FINW"],
        "RWP": folded["RWP"], "RWD": folded["RWD"],
        "W2L": folded["W2L"], "W3L": folded["W3L"],
    }
    in_maps = []
    for c in range(NCORES):
        s = slice(c * BCORE, (c + 1) * BCORE)
        m = dict(shared)
        xrT = np.ascontiguousarray(xr[s].T).astype(bf16)
        xiT = np.ascontiguousarray(xi[s].T).astype(bf16)
        xri = np.empty((ROWS, 2, BCORE), bf16)
        xri[:, 0, :] = xrT
        xri[:, 1, :] = xiT
        m["xri"] = xri
        xm = np.empty((NO, 2, BCORE), bf16)
        xm[0:50, 0, :] = xrT[0:50]
        xm[50:100, 0, :] = xrT[0:50]
        xm[0:50, 1, :] = xiT[0:50]
        xm[50:100, 1, :] = xiT[0:50]
        m["xmri"] = xm
        m["pex8"] = np.ascontiguousarray(
            np.broadcast_to(P[s][None, :], (8, BCORE))).astype(bf16)
        ext = np.empty((4, BCORE), np.float32)
        ext[0] = xr[s][:, 2 * LH]       # center tap re, pol 0
        ext[1] = xr[s][:, 2 * LH + 1]   # pol 1
        ext[2] = xi[s][:, 2 * LH]
        ext[3] = xi[s][:, 2 * LH + 1]
        m["ext"] = ext
        in_maps.append(m)
    return in_maps


_CACHE = {}


def kernel(**inputs):
    from concourse.bass_utils import run_bass_kernel_spmd

    static = build_static()
    folded = fold_weights(
        np.asarray(inputs["W1_real"]), np.asarray(inputs["W1_imag"]),
        np.asarray(inputs["W2_real"]), np.asarray(inputs["W2_imag"]),
        np.asarray(inputs["W3_real"]), np.asarray(inputs["W3_imag"]),
    )
    if "nc" not in _CACHE:
        _CACHE["nc"] = build_nc()
    nc = _CACHE["nc"]
    in_maps = _prep_core_inputs(inputs, static, folded)
    res = run_bass_kernel_spmd(nc, in_maps, list(range(NCORES)))
    nseg = NCHUNK // 4
    out = np.empty((BATCH, NM, 2), np.float32)
    for c in range(NCORES):
        o4 = np.concatenate([res.results[c][f"out{j}"] for j in range(nseg)], axis=1)
        s = slice(c * BCORE, (c + 1) * BCORE)
        out[s, 0, 0] = o4[0]
        out[s, 1, 0] = o4[1]
        out[s, 0, 1] = o4[2]
        out[s, 1, 1] = o4[3]
    return out


# revision 11
# speedup vs baseline: 51.3160x; 3.5448x over previous
"""EqPBCNN (perturbation-based nonlinearity compensation NN) Trainium2 Bass kernel.

Data-parallel over 8 NeuronCores: batch 65536 -> 8192 per core.

Math (per sample, per polarization p):
  S_h = sum_p' x[n_h, p'] * conj(x[m_h+n_h, p'])        (pol-summed pair product)
  R[p,o,m] = sum_{n:(m,n)} W1'[p,o,(m,n)] * S_(m,n)
  h1[p,o] = sum_m x[m,p] * R[p,o,m]
  h1 -> CLrelu -> W2 -> CLrelu -> W3 -> E;  out = x[center,p] + E * 10^(task0/10)/2

Structural reductions vs the direct formulation:
  * the 175 (m,n) triplets collapse to 148 distinct unordered tap pairs:
    127 complex off-diagonal pairs (2 x 128-row PE splits) and 21 purely-real
    diagonal pairs |x_t|^2 computed from contiguous x rows (no gather).
  * conjugate-mirror duplicates fold into the R weights as sign flips.
  * the m-value set is contiguous (-20..4): the x_m operand is x[0:50]
    replicated, shipped pre-replicated from HBM.
  * re/im are packed side by side on the free dim, so each gather matmul
    pair lands in ONE combined PSUM tile drained by ONE copy (PE cannot
    read PSUM; only ACT/DVE can, so drain count is the scarce resource).
  * P = 10^(task/10) is precomputed on host and folded through the MLP
    via one multiply on h1 (leaky_relu is positively homogeneous).
  * all products/adds run on bf16 SBUF tiles (Pool cannot access PSUM).
"""
import numpy as np

# ---------------- problem constants (hardcoded; must match reference) -------
BATCH = 65536
MT, LH = 41, 20          # filter taps, half window
NM = 2                   # modes / polarizations
H1, H2 = 2, 10
SLOPE = 0.01
NCORES = 8
BCORE = BATCH // NCORES  # 8192
NB = 512                 # samples per chunk
NCHUNK = BCORE // NB     # 16
ROWS = MT * NM           # 82 = tap*2 + mode

_idx = [(m, n) for m in range(-LH, LH + 1) for n in range(-LH, LH + 1)
        if abs(m * n) <= LH and abs(m + n) <= LH and n >= m]
H = len(_idx)            # 175
M_ARR = np.array([t[0] for t in _idx], np.int32)
N_ARR = np.array([t[1] for t in _idx], np.int32)
SYM = np.where(M_ARR != N_ARR, 2.0, 1.0).astype(np.float32)
M_VALS = list(range(-20, 5))             # the 25 distinct m values (contiguous)
NMV = len(M_VALS)
NO = H1 * NMV * NM       # 100 rows of R/T space: (o, mi, p) o-major

# canonical unordered pair set
_canon = {}
PAIR_INFO = []           # per h: ('d', n) diagonal | ('c', k, conj_flag)
for _h, (_m, _n) in enumerate(_idx):
    _a, _b = _n + LH, _m + _n + LH
    if _a == _b:
        PAIR_INFO.append(('d', _a - LH))
    else:
        _key = (min(_a, _b), max(_a, _b))
        if _key not in _canon:
            _canon[_key] = len(_canon)
        PAIR_INFO.append(('c', _canon[_key], _a > _b))
NOFF = len(_canon)                       # 127 off-diagonal pairs
A_K = np.array([k[0] for k in _canon], np.int32)   # smaller tap
B_K = np.array([k[1] for k in _canon], np.int32)   # larger tap (conjugated)
NSTACK = 2 * NOFF                        # 254 rows (pair, pol)
KSPLITS = [(0, 128), (128, NSTACK - 128)]          # 128 + 126
NDIAG = ROWS                             # diag power rows: full x tile; rows 40..81 weighted
CONSTCOLS = 1584                         # packed consts: SEL|RWP0|RWP1|RWD|FINW|W2L|W3L


def pack_consts(static, folded):
    C = np.zeros((128, CONSTCOLS), np.float32)
    C[0:ROWS, 0:508] = static["SEL"]
    C[0:128, 508:908] = folded["RWP"][0]
    C[0:128, 908:1308] = folded["RWP"][1]
    C[0:NDIAG, 1308:1508] = folded["RWD"]
    C[0:NO, 1508:1540] = static["FINW"]
    C[0:8, 1540:1580] = folded["W2L"]
    C[0:40, 1580:1584] = folded["W3L"]
    return C


def _orow(o, mi, p):
    return o * (NMV * NM) + 2 * mi + p   # o-major: rows 0:50 o=0, 50:100 o=1


def _hrow(p, o, comp):
    return (p * H1 + o) * 2 + comp


def _h2row(p, q, comp):
    return (p * H2 + q) * 2 + comp


def build_static():
    """Weight-independent constant matrices."""
    # gather selections: cols 0:254 A-side (tap a_k), 254:508 C-side (tap b_k)
    SEL = np.zeros((ROWS, 2 * NSTACK), np.float32)
    for k in range(NOFF):
        for p in range(NM):
            SEL[2 * A_K[k] + p, 2 * k + p] = 1.0
            SEL[2 * B_K[k] + p, NSTACK + 2 * k + p] = 1.0
    # final contraction [100, 32]: h1re = sum(u1) - sum(u2); h1im = sum(u3) + sum(u4)
    FINW = np.zeros((NO, 32), np.float32)
    for o in range(H1):
        for mi in range(NMV):
            for p in range(NM):
                FINW[_orow(o, mi, p), 0 + _hrow(p, o, 0)] = 1.0
                FINW[_orow(o, mi, p), 8 + _hrow(p, o, 0)] = -1.0
                FINW[_orow(o, mi, p), 16 + _hrow(p, o, 1)] = 1.0
                FINW[_orow(o, mi, p), 24 + _hrow(p, o, 1)] = 1.0
    return {"SEL": SEL, "FINW": FINW}


def fold_weights(W1r, W1i, W2r, W2i, W3r, W3i):
    """Runtime weight folding into matmul lhsT constants."""
    Wr = W1r * SYM[None, None, :]   # [p, o, h]
    Wi = W1i * SYM[None, None, :]
    # R lhsT for off-diag G stacks: per split s, [128, 400]
    #   cols 0:100 Gre->Rre, 100:200 Gim->Rre, 200:300 Gre->Rim, 300:400 Gim->Rim
    RW = np.zeros((NSTACK, 4 * NO), np.float32)
    # diag-power lhsT [82, 200]: 0:100 pw->Rre, 100:200 pw->Rim (rows 40..81 live)
    RWD = np.zeros((NDIAG, 2 * NO), np.float32)
    for h in range(H):
        mi = M_VALS.index(M_ARR[h])
        info = PAIR_INFO[h]
        for p in range(NM):
            for o in range(H1):
                c = _orow(o, mi, p)
                wre, wim = Wr[p, o, h], Wi[p, o, h]
                if info[0] == 'd':
                    n = info[1]
                    for q in range(NM):
                        RWD[40 + 2 * n + q, c] += wre
                        RWD[40 + 2 * n + q, NO + c] += wim
                else:
                    _, k, flag = info
                    sgn = -1.0 if flag else 1.0   # S = conj(q) when flag
                    for q in range(NM):
                        r = 2 * k + q
                        RW[r, 0 * NO + c] += wre
                        RW[r, 1 * NO + c] -= wim * sgn
                        RW[r, 2 * NO + c] += wim
                        RW[r, 3 * NO + c] += wre * sgn
    RWP = np.zeros((2, 128, 4 * NO), np.float32)
    for s, (r0, rk) in enumerate(KSPLITS):
        RWP[s, :rk, :] = RW[r0:r0 + rk, :]
    # W2 lhsT [8, 40]
    W2L = np.zeros((8, 2 * H2 * NM), np.float32)
    for p in range(NM):
        for q in range(H2):
            for o in range(H1):
                W2L[_hrow(p, o, 0), _h2row(p, q, 0)] += W2r[p, q, o]
                W2L[_hrow(p, o, 1), _h2row(p, q, 0)] -= W2i[p, q, o]
                W2L[_hrow(p, o, 0), _h2row(p, q, 1)] += W2i[p, q, o]
                W2L[_hrow(p, o, 1), _h2row(p, q, 1)] += W2r[p, q, o]
    # W3 lhsT [40, 4]: cols [re_p0, re_p1, im_p0, im_p1]; 1/NM folded
    W3L = np.zeros((2 * H2 * NM, 4), np.float32)
    s3 = 1.0 / NM
    for p in range(NM):
        for q in range(H2):
            W3L[_h2row(p, q, 0), 0 + p] += W3r[p, 0, q] * s3
            W3L[_h2row(p, q, 1), 0 + p] -= W3i[p, 0, q] * s3
            W3L[_h2row(p, q, 0), 2 + p] += W3i[p, 0, q] * s3
            W3L[_h2row(p, q, 1), 2 + p] += W3r[p, 0, q] * s3
    return {"RWP": RWP, "RWD": RWD, "W2L": W2L, "W3L": W3L}


# ---------------------------------------------------------------------------
def build_nc(bcore=BCORE, nb=NB, lrelu_mode="act"):
    """Build the Bass program for one core processing `bcore` samples."""
    import concourse.bass as bass
    import concourse.bacc as bacc
    import concourse.mybir as mybir
    from concourse.tile import TileContext
    import bass_rust

    nchunk = bcore // nb
    assert nchunk * nb == bcore
    grp = 4 if nchunk % 4 == 0 else 1
    f32 = mybir.dt.float32
    bf16 = mybir.dt.bfloat16
    AF = bass_rust.ActivationFunctionType
    OP = mybir.AluOpType

    nc = bacc.Bacc(None, target_bir_lowering=False, debug=False)
    # all bf16 sample data in one tensor: rows 0:82 x (re||im), 82:182 xrep,
    # 182:190 P replicated; re/im side by side on the middle dim
    xD = nc.declare_dram_parameter("xall", [190, 2, bcore], bf16, isOutput=False)
    extD = nc.declare_dram_parameter("ext", [4, bcore], f32, isOutput=False)
    # all folded weights/constants packed into one [128, 1584] f32 tensor
    cD = nc.declare_dram_parameter("consts", [128, CONSTCOLS], f32, isOutput=False)
    outD = nc.declare_dram_parameter("out", [4, bcore], f32, isOutput=True)

    with TileContext(nc) as tc:
        with (
            tc.tile_pool(name="consts", bufs=1) as cp,
            tc.tile_pool(name="xt", bufs=3) as xp,
            tc.tile_pool(name="g", bufs=2) as gp,
            tc.tile_pool(name="tmp", bufs=2) as tp,
            tc.tile_pool(name="small", bufs=3) as sp,
            tc.tile_pool(name="psum", bufs=1, space="PSUM") as pp,
        ):
            c32 = cp.tile([128, CONSTCOLS], f32, name="c32")
            nc.gpsimd.dma_start(out=c32[:], in_=cD[:])

            def const_tile(rows, c0, c1, name):
                tr = cp.tile([rows, c1 - c0], bf16, name=name)
                nc.vector.tensor_copy(tr[:], c32[0:rows, c0:c1])
                return tr

            sel_sb = const_tile(ROWS, 0, 508, "sel")
            rw_sb = [const_tile(128, 508 + 400 * s, 908 + 400 * s, f"rw{s}")
                     for s in range(2)]
            rwd_sb = const_tile(NDIAG, 1308, 1508, "rwd")
            fin_sb = const_tile(NO, 1508, 1540, "fin")
            w2_sb = const_tile(8, 1540, 1580, "w2")
            w3_sb = const_tile(40, 1580, 1584, "w3")

            # ---------------- software-pipelined chunk stages ----------------
            # PE issue order per iteration i: gathers(i), R(i-1), fin/w2/w3(i-2)
            # so PE never waits on the drain->product chain or the tail chain.
            def lrelu(dst, src, rows):
                if lrelu_mode == "act":
                    nc.scalar.activation(dst[:rows], src[:rows], AF.Lrelu, alpha=SLOPE)
                else:
                    nc.vector.tensor_scalar_mul(dst[:rows], src[:rows], SLOPE)
                    nc.vector.tensor_tensor(dst[:rows], dst[:rows], src[:rows], op=OP.max)

            grp_tiles = {}
            st = {}     # per-chunk live tiles

            def stage_load(c):
                cs = slice(c * nb, (c + 1) * nb)
                xt = xp.tile([ROWS, 2, nb], bf16, tag="xt", bufs=3)
                xm = xp.tile([NO, 2, nb], bf16, tag="xm", bufs=3)
                px8 = xp.tile([8, nb], bf16, tag="px8", bufs=3)
                nc.sync.dma_start(out=xt[:], in_=xD[0:ROWS, :, cs])
                nc.sync.dma_start(out=xm[:], in_=xD[ROWS:ROWS + NO, :, cs])
                nc.sync.dma_start(out=px8[:], in_=xD[182:190, 0, cs])
                if c % grp == 0:
                    cs4 = slice(c * nb, (c + grp) * nb)
                    o_re2 = sp.tile([2, grp * nb], f32, tag="ore", bufs=max(1, nchunk // grp), name="o_re2")
                    o_im2 = sp.tile([2, grp * nb], f32, tag="oim", bufs=max(1, nchunk // grp), name="o_im2")
                    nc.sync.dma_start(out=o_re2[:], in_=extD[0:2, cs4])
                    nc.sync.dma_start(out=o_im2[:], in_=extD[2:4, cs4])
                    grp_tiles[c // grp] = (o_re2, o_im2)
                st[c] = {"xt": xt, "xm": xm, "px8": px8}

            def stage_gather(c):
                xt = st[c]["xt"]
                pas, pcs = [], []
                for s, (r0, rk) in enumerate(KSPLITS):
                    pa = pp.tile([128, 2, nb], f32, tag="pa", bufs=1)
                    pc = pp.tile([128, 2, nb], f32, tag="pc", bufs=1)
                    a_sl = sel_sb[:, r0:r0 + rk]
                    c_sl = sel_sb[:, NSTACK + r0:NSTACK + r0 + rk]
                    nc.tensor.matmul(pa[:rk, 0, :], a_sl, xt[:, 0, :], start=True, stop=True)
                    nc.tensor.matmul(pa[:rk, 1, :], a_sl, xt[:, 1, :], start=True, stop=True)
                    nc.tensor.matmul(pc[:rk, 0, :], c_sl, xt[:, 0, :], start=True, stop=True)
                    nc.tensor.matmul(pc[:rk, 1, :], c_sl, xt[:, 1, :], start=True, stop=True)
                    pas.append(pa)
                    pcs.append(pc)
                st[c]["pa"] = pas
                st[c]["pc"] = pcs

            def stage_drain(c):
                dr = []
                for s, (r0, rk) in enumerate(KSPLITS):
                    pa, pc = st[c]["pa"][s], st[c]["pc"][s]
                    aS = tp.tile([128, 2, nb], bf16, tag="aS", bufs=2)
                    cS = tp.tile([128, 2, nb], bf16, tag="cS", bufs=2)
                    if s == 0:
                        nc.scalar.copy(aS[:rk], pa[:rk])
                    else:
                        nc.vector.tensor_copy(aS[:rk], pa[:rk])
                    nc.scalar.copy(cS[:rk], pc[:rk])
                    dr.append((aS, cS))
                st[c]["dr"] = dr

            def stage_products(c):
                xt = st[c]["xt"]
                # diagonal power rows: pw = xre^2 + xim^2 (rows 40..81 weighted)
                ud = sp.tile([NDIAG, 2, nb], bf16, tag="ud", bufs=2)
                pw = gp.tile([NDIAG, nb], bf16, tag="pw", bufs=2)
                nc.gpsimd.tensor_tensor(ud[:], xt[:], xt[:], op=OP.mult)
                nc.vector.tensor_tensor(pw[:], ud[:, 0, :], ud[:, 1, :], op=OP.add)
                g_tiles = []
                for s, (r0, rk) in enumerate(KSPLITS):
                    aS, cS = st[c]["dr"][s]
                    m1 = tp.tile([128, nb], bf16, tag="m1", bufs=2)
                    m2 = tp.tile([128, nb], bf16, tag="m2", bufs=2)
                    m3 = tp.tile([128, nb], bf16, tag="m3", bufs=2)
                    m4 = tp.tile([128, nb], bf16, tag="m4", bufs=2)
                    gr = gp.tile([128, nb], bf16, tag=f"gr{s}")
                    gi = gp.tile([128, nb], bf16, tag=f"gi{s}")
                    nc.gpsimd.tensor_tensor(m1[:rk], aS[:rk, 0, :], cS[:rk, 0, :], op=OP.mult)
                    nc.gpsimd.tensor_tensor(m2[:rk], aS[:rk, 1, :], cS[:rk, 1, :], op=OP.mult)
                    nc.gpsimd.tensor_tensor(m3[:rk], aS[:rk, 1, :], cS[:rk, 0, :], op=OP.mult)
                    nc.gpsimd.tensor_tensor(m4[:rk], aS[:rk, 0, :], cS[:rk, 1, :], op=OP.mult)
                    nc.vector.tensor_tensor(gr[:rk], m1[:rk], m2[:rk], op=OP.add)
                    nc.vector.tensor_tensor(gi[:rk], m3[:rk], m4[:rk], op=OP.subtract)
                    g_tiles.append((gr, gi))
                st[c]["g"] = g_tiles
                st[c]["pw"] = pw

            def stage_R(c):
                g_tiles, pw, xm = st[c]["g"], st[c]["pw"], st[c]["xm"]
                p_R = pp.tile([128, 2, nb], f32, tag="pR", bufs=1)
                for s, (r0, rk) in enumerate(KSPLITS):
                    gr, gi = g_tiles[s]
                    rw = rw_sb[s]
                    nc.tensor.matmul(p_R[:NO, 0, :], rw[:rk, 0:NO], gr[:rk],
                                     start=(s == 0), stop=False)
                    nc.tensor.matmul(p_R[:NO, 0, :], rw[:rk, NO:2 * NO], gi[:rk],
                                     start=False, stop=False)
                    nc.tensor.matmul(p_R[:NO, 1, :], rw[:rk, 2 * NO:3 * NO], gr[:rk],
                                     start=(s == 0), stop=False)
                    nc.tensor.matmul(p_R[:NO, 1, :], rw[:rk, 3 * NO:4 * NO], gi[:rk],
                                     start=False, stop=False)
                nc.tensor.matmul(p_R[:NO, 0, :], rwd_sb[:, 0:NO], pw[:], start=False, stop=True)
                nc.tensor.matmul(p_R[:NO, 1, :], rwd_sb[:, NO:2 * NO], pw[:], start=False, stop=True)
                rS = tp.tile([NO, 2, nb], bf16, tag="rS", bufs=2)
                nc.scalar.copy(rS[:], p_R[:NO])
                # T products (xrep * R), contracted by fin with +- weights next iter
                u1 = tp.tile([NO, nb], bf16, tag="u1", bufs=2)
                u2 = tp.tile([NO, nb], bf16, tag="u2", bufs=2)
                u3 = tp.tile([NO, nb], bf16, tag="u3", bufs=2)
                u4 = tp.tile([NO, nb], bf16, tag="u4", bufs=2)
                nc.gpsimd.tensor_tensor(u1[:], xm[:, 0, :], rS[:, 0, :], op=OP.mult)
                nc.gpsimd.tensor_tensor(u2[:], xm[:, 1, :], rS[:, 1, :], op=OP.mult)
                nc.vector.tensor_tensor(u3[:], xm[:, 0, :], rS[:, 1, :], op=OP.mult)
                nc.vector.tensor_tensor(u4[:], xm[:, 1, :], rS[:, 0, :], op=OP.mult)
                st[c]["u"] = (u1, u2, u3, u4)

            def stage_tail(c):
                u1, u2, u3, u4 = st[c]["u"]
                px8 = st[c]["px8"]
                p_h1 = pp.tile([128, nb], f32, tag="ph1", bufs=1)
                nc.tensor.matmul(p_h1[:8], fin_sb[:, 0:8], u1[:], start=True, stop=False)
                nc.tensor.matmul(p_h1[:8], fin_sb[:, 8:16], u2[:], start=False, stop=False)
                nc.tensor.matmul(p_h1[:8], fin_sb[:, 16:24], u3[:], start=False, stop=False)
                nc.tensor.matmul(p_h1[:8], fin_sb[:, 24:32], u4[:], start=False, stop=True)
                h1s = sp.tile([8, nb], bf16, tag="h1s")
                lrelu(h1s, p_h1, 8)
                # P > 0 and lrelu is positively homogeneous: fold P into h1
                h1p = sp.tile([8, nb], bf16, tag="h1p")
                nc.vector.tensor_tensor(h1p[:], h1s[:], px8[:], op=OP.mult)
                p_h2 = pp.tile([128, nb], f32, tag="misc", bufs=1)
                nc.tensor.matmul(p_h2[:40], w2_sb[:], h1p[:8], start=True, stop=True)
                h2s = sp.tile([40, nb], bf16, tag="h2s")
                lrelu(h2s, p_h2, 40)
                p_ere = pp.tile([128, nb], f32, tag="misc", bufs=1)
                p_eim = pp.tile([128, nb], f32, tag="misc", bufs=1)
                nc.tensor.matmul(p_ere[:2], w3_sb[:, 0:2], h2s[:], start=True, stop=True)
                nc.tensor.matmul(p_eim[:2], w3_sb[:, 2:4], h2s[:], start=True, stop=True)
                # out = center + E' (P already folded into E')
                half = (c % grp) * nb
                o_re = grp_tiles[c // grp][0][:, half:half + nb]
                o_im = grp_tiles[c // grp][1][:, half:half + nb]
                nc.vector.tensor_tensor(o_re, p_ere[:2], o_re, op=OP.add)
                nc.vector.tensor_tensor(o_im, p_eim[:2], o_im, op=OP.add)
                if c % grp == grp - 1:
                    g0 = c // grp
                    gsl = slice(g0 * grp * nb, (g0 + 1) * grp * nb)
                    nc.sync.dma_start(out=outD[0:2, gsl], in_=grp_tiles[g0][0][:])
                    nc.sync.dma_start(out=outD[2:4, gsl], in_=grp_tiles[g0][1][:])
                del st[c]

            for i in range(nchunk + 2):
                if i < nchunk:
                    stage_load(i)
                    stage_gather(i)          # PE
                if i >= 1 and i - 1 < nchunk:
                    stage_R(i - 1)           # PE + rS/u
                if i >= 2:
                    stage_tail(i - 2)        # PE fin + MLP tail
                if i < nchunk:
                    stage_drain(i)           # ACT/DVE
                    stage_products(i)        # POOL/DVE
    nc.compile()
    return nc


def _prep_core_inputs(inputs, static, folded):
    """Shard + lay out inputs per core. Returns list of in_maps."""
    import ml_dtypes
    bf16 = ml_dtypes.bfloat16
    xr = np.ascontiguousarray(inputs["x_real"]).reshape(BATCH, ROWS)
    xi = np.ascontiguousarray(inputs["x_imag"]).reshape(BATCH, ROWS)
    t0 = np.ascontiguousarray(inputs["task_info"][:, 0])
    P = (10.0 ** (t0.astype(np.float64) / 10.0)).astype(np.float32)
    consts = pack_consts(static, folded)
    in_maps = []
    for c in range(NCORES):
        s = slice(c * BCORE, (c + 1) * BCORE)
        m = {"consts": consts}
        xrT = np.ascontiguousarray(xr[s].T).astype(bf16)
        xiT = np.ascontiguousarray(xi[s].T).astype(bf16)
        Pb = P[s].astype(bf16)
        xall = np.empty((190, 2, BCORE), bf16)
        xall[0:ROWS, 0, :] = xrT
        xall[0:ROWS, 1, :] = xiT
        xall[ROWS:ROWS + 50, 0, :] = xrT[0:50]
        xall[ROWS + 50:ROWS + 100, 0, :] = xrT[0:50]
        xall[ROWS:ROWS + 50, 1, :] = xiT[0:50]
        xall[ROWS + 50:ROWS + 100, 1, :] = xiT[0:50]
        xall[182:190, 0, :] = Pb[None, :]
        xall[182:190, 1, :] = 0
        m["xall"] = xall
        ext = np.empty((4, BCORE), np.float32)
        ext[0] = xr[s][:, 2 * LH]       # center tap re, pol 0
        ext[1] = xr[s][:, 2 * LH + 1]   # pol 1
        ext[2] = xi[s][:, 2 * LH]
        ext[3] = xi[s][:, 2 * LH + 1]
        m["ext"] = ext
        in_maps.append(m)
    return in_maps


_CACHE = {}


def kernel(**inputs):
    from concourse.bass_utils import run_bass_kernel_spmd

    static = build_static()
    folded = fold_weights(
        np.asarray(inputs["W1_real"]), np.asarray(inputs["W1_imag"]),
        np.asarray(inputs["W2_real"]), np.asarray(inputs["W2_imag"]),
        np.asarray(inputs["W3_real"]), np.asarray(inputs["W3_imag"]),
    )
    if "nc" not in _CACHE:
        _CACHE["nc"] = build_nc()
    nc = _CACHE["nc"]
    in_maps = _prep_core_inputs(inputs, static, folded)
    res = run_bass_kernel_spmd(nc, in_maps, list(range(NCORES)))
    out = np.empty((BATCH, NM, 2), np.float32)
    for c in range(NCORES):
        o4 = res.results[c]["out"]
        s = slice(c * BCORE, (c + 1) * BCORE)
        out[s, 0, 0] = o4[0]
        out[s, 1, 0] = o4[1]
        out[s, 0, 1] = o4[2]
        out[s, 1, 1] = o4[3]
    return out


# revision 12
# speedup vs baseline: 54.8510x; 1.0689x over previous
"""EqPBCNN (perturbation-based nonlinearity compensation NN) Trainium2 Bass kernel.

Data-parallel over 8 NeuronCores: batch 65536 -> 8192 per core.

Math (per sample, per polarization p):
  S_h = sum_p' x[n_h, p'] * conj(x[m_h+n_h, p'])        (pol-summed pair product)
  R[p,o,m] = sum_{n:(m,n)} W1'[p,o,(m,n)] * S_(m,n)
  h1[p,o] = sum_m x[m,p] * R[p,o,m]
  h1 -> CLrelu -> W2 -> CLrelu -> W3 -> E;  out = x[center,p] + E * 10^(task0/10)/2

Structural reductions vs the direct formulation:
  * the 175 (m,n) triplets collapse to 148 distinct unordered tap pairs:
    127 complex off-diagonal pairs (2 x 128-row PE splits) and 21 purely-real
    diagonal pairs |x_t|^2 computed from contiguous x rows (no gather).
  * conjugate-mirror duplicates fold into the R weights as sign flips.
  * the m-value set is contiguous (-20..4): the x_m operand is x[0:50]
    replicated, shipped pre-replicated from HBM.
  * re/im are packed side by side on the free dim, so each gather matmul
    pair lands in ONE combined PSUM tile drained by ONE copy (PE cannot
    read PSUM; only ACT/DVE can, so drain count is the scarce resource).
  * P = 10^(task/10) is precomputed on host and folded through the MLP
    via one multiply on h1 (leaky_relu is positively homogeneous).
  * all products/adds run on bf16 SBUF tiles (Pool cannot access PSUM).
"""
import numpy as np

# ---------------- problem constants (hardcoded; must match reference) -------
BATCH = 65536
MT, LH = 41, 20          # filter taps, half window
NM = 2                   # modes / polarizations
H1, H2 = 2, 10
SLOPE = 0.01
NCORES = 8
BCORE = BATCH // NCORES  # 8192
NB = 512                 # samples per chunk
NCHUNK = BCORE // NB     # 16
ROWS = MT * NM           # 82 = tap*2 + mode

_idx = [(m, n) for m in range(-LH, LH + 1) for n in range(-LH, LH + 1)
        if abs(m * n) <= LH and abs(m + n) <= LH and n >= m]
H = len(_idx)            # 175
M_ARR = np.array([t[0] for t in _idx], np.int32)
N_ARR = np.array([t[1] for t in _idx], np.int32)
SYM = np.where(M_ARR != N_ARR, 2.0, 1.0).astype(np.float32)
M_VALS = list(range(-20, 5))             # the 25 distinct m values (contiguous)
NMV = len(M_VALS)
NO = H1 * NMV * NM       # 100 rows of R/T space: (o, mi, p) o-major

# canonical unordered pair set
_canon = {}
PAIR_INFO = []           # per h: ('d', n) diagonal | ('c', k, conj_flag)
for _h, (_m, _n) in enumerate(_idx):
    _a, _b = _n + LH, _m + _n + LH
    if _a == _b:
        PAIR_INFO.append(('d', _a - LH))
    else:
        _key = (min(_a, _b), max(_a, _b))
        if _key not in _canon:
            _canon[_key] = len(_canon)
        PAIR_INFO.append(('c', _canon[_key], _a > _b))
NOFF = len(_canon)                       # 127 off-diagonal pairs
A_K = np.array([k[0] for k in _canon], np.int32)   # smaller tap
B_K = np.array([k[1] for k in _canon], np.int32)   # larger tap (conjugated)
NSTACK = 2 * NOFF                        # 254 rows (pair, pol)
KSPLITS = [(0, 128), (128, NSTACK - 128)]          # 128 + 126
NDIAG = ROWS                             # diag power rows: full x tile; rows 40..81 weighted
CONSTCOLS = 1584                         # packed consts: SEL|RWP0|RWP1|RWD|FINW|W2L|W3L


def pack_consts(static, folded):
    C = np.zeros((128, CONSTCOLS), np.float32)
    C[0:ROWS, 0:508] = static["SEL"]
    C[0:128, 508:908] = folded["RWP"][0]
    C[0:128, 908:1308] = folded["RWP"][1]
    C[0:NDIAG, 1308:1508] = folded["RWD"]
    C[0:NO, 1508:1540] = static["FINW"]
    C[0:8, 1540:1580] = folded["W2L"]
    C[0:40, 1580:1584] = folded["W3L"]
    return C


def _orow(o, mi, p):
    return o * (NMV * NM) + 2 * mi + p   # o-major: rows 0:50 o=0, 50:100 o=1


def _hrow(p, o, comp):
    return (p * H1 + o) * 2 + comp


def _h2row(p, q, comp):
    return (p * H2 + q) * 2 + comp


def build_static():
    """Weight-independent constant matrices."""
    # gather selections: cols 0:254 A-side (tap a_k), 254:508 C-side (tap b_k)
    SEL = np.zeros((ROWS, 2 * NSTACK), np.float32)
    for k in range(NOFF):
        for p in range(NM):
            SEL[2 * A_K[k] + p, 2 * k + p] = 1.0
            SEL[2 * B_K[k] + p, NSTACK + 2 * k + p] = 1.0
    # final contraction [100, 32]: h1re = sum(u1) - sum(u2); h1im = sum(u3) + sum(u4)
    FINW = np.zeros((NO, 32), np.float32)
    for o in range(H1):
        for mi in range(NMV):
            for p in range(NM):
                FINW[_orow(o, mi, p), 0 + _hrow(p, o, 0)] = 1.0
                FINW[_orow(o, mi, p), 8 + _hrow(p, o, 0)] = -1.0
                FINW[_orow(o, mi, p), 16 + _hrow(p, o, 1)] = 1.0
                FINW[_orow(o, mi, p), 24 + _hrow(p, o, 1)] = 1.0
    return {"SEL": SEL, "FINW": FINW}


def fold_weights(W1r, W1i, W2r, W2i, W3r, W3i):
    """Runtime weight folding into matmul lhsT constants."""
    Wr = W1r * SYM[None, None, :]   # [p, o, h]
    Wi = W1i * SYM[None, None, :]
    # R lhsT for off-diag G stacks: per split s, [128, 400]
    #   cols 0:100 Gre->Rre, 100:200 Gim->Rre, 200:300 Gre->Rim, 300:400 Gim->Rim
    RW = np.zeros((NSTACK, 4 * NO), np.float32)
    # diag-power lhsT [82, 200]: 0:100 pw->Rre, 100:200 pw->Rim (rows 40..81 live)
    RWD = np.zeros((NDIAG, 2 * NO), np.float32)
    for h in range(H):
        mi = M_VALS.index(M_ARR[h])
        info = PAIR_INFO[h]
        for p in range(NM):
            for o in range(H1):
                c = _orow(o, mi, p)
                wre, wim = Wr[p, o, h], Wi[p, o, h]
                if info[0] == 'd':
                    n = info[1]
                    for q in range(NM):
                        RWD[40 + 2 * n + q, c] += wre
                        RWD[40 + 2 * n + q, NO + c] += wim
                else:
                    _, k, flag = info
                    sgn = -1.0 if flag else 1.0   # S = conj(q) when flag
                    for q in range(NM):
                        r = 2 * k + q
                        RW[r, 0 * NO + c] += wre
                        RW[r, 1 * NO + c] -= wim * sgn
                        RW[r, 2 * NO + c] += wim
                        RW[r, 3 * NO + c] += wre * sgn
    RWP = np.zeros((2, 128, 4 * NO), np.float32)
    for s, (r0, rk) in enumerate(KSPLITS):
        RWP[s, :rk, :] = RW[r0:r0 + rk, :]
    # W2 lhsT [8, 40]
    W2L = np.zeros((8, 2 * H2 * NM), np.float32)
    for p in range(NM):
        for q in range(H2):
            for o in range(H1):
                W2L[_hrow(p, o, 0), _h2row(p, q, 0)] += W2r[p, q, o]
                W2L[_hrow(p, o, 1), _h2row(p, q, 0)] -= W2i[p, q, o]
                W2L[_hrow(p, o, 0), _h2row(p, q, 1)] += W2i[p, q, o]
                W2L[_hrow(p, o, 1), _h2row(p, q, 1)] += W2r[p, q, o]
    # W3 lhsT [40, 4]: cols [re_p0, re_p1, im_p0, im_p1]; 1/NM folded
    W3L = np.zeros((2 * H2 * NM, 4), np.float32)
    s3 = 1.0 / NM
    for p in range(NM):
        for q in range(H2):
            W3L[_h2row(p, q, 0), 0 + p] += W3r[p, 0, q] * s3
            W3L[_h2row(p, q, 1), 0 + p] -= W3i[p, 0, q] * s3
            W3L[_h2row(p, q, 0), 2 + p] += W3i[p, 0, q] * s3
            W3L[_h2row(p, q, 1), 2 + p] += W3r[p, 0, q] * s3
    return {"RWP": RWP, "RWD": RWD, "W2L": W2L, "W3L": W3L}


# ---------------------------------------------------------------------------
def build_nc(bcore=BCORE, nb=NB, lrelu_mode="act"):
    """Build the Bass program for one core processing `bcore` samples."""
    import concourse.bass as bass
    import concourse.bacc as bacc
    import concourse.mybir as mybir
    from concourse.tile import TileContext
    import bass_rust

    nchunk = bcore // nb
    assert nchunk * nb == bcore
    grp = 4 if nchunk % 4 == 0 else 1
    f32 = mybir.dt.float32
    bf16 = mybir.dt.bfloat16
    AF = bass_rust.ActivationFunctionType
    OP = mybir.AluOpType

    nc = bacc.Bacc(None, target_bir_lowering=False, debug=False)
    # all bf16 sample data in one tensor: rows 0:82 x (re||im), 82:182 xrep,
    # 182:190 P replicated; re/im side by side on the middle dim
    xD = nc.declare_dram_parameter("xall", [190, 2, bcore], bf16, isOutput=False)
    extD = nc.declare_dram_parameter("ext", [4, bcore], f32, isOutput=False)
    # all folded weights/constants packed into one [128, 1584] f32 tensor
    cD = nc.declare_dram_parameter("consts", [128, CONSTCOLS], f32, isOutput=False)
    outD = nc.declare_dram_parameter("out", [4, bcore], f32, isOutput=True)

    with TileContext(nc) as tc:
        with (
            tc.tile_pool(name="consts", bufs=1) as cp,
            tc.tile_pool(name="xt", bufs=3) as xp,
            tc.tile_pool(name="g", bufs=2) as gp,
            tc.tile_pool(name="tmp", bufs=2) as tp,
            tc.tile_pool(name="small", bufs=3) as sp,
            tc.tile_pool(name="psum", bufs=1, space="PSUM") as pp,
        ):
            c32 = cp.tile([128, CONSTCOLS], f32, name="c32")
            nc.gpsimd.dma_start(out=c32[:], in_=cD[:])

            def const_tile(rows, c0, c1, name):
                tr = cp.tile([rows, c1 - c0], bf16, name=name)
                nc.vector.tensor_copy(tr[:], c32[0:rows, c0:c1])
                return tr

            sel_sb = const_tile(ROWS, 0, 508, "sel")
            rw_sb = [const_tile(128, 508 + 400 * s, 908 + 400 * s, f"rw{s}")
                     for s in range(2)]
            rwd_sb = const_tile(NDIAG, 1308, 1508, "rwd")
            fin_sb = const_tile(NO, 1508, 1540, "fin")
            w2_sb = const_tile(8, 1540, 1580, "w2")
            w3_sb = const_tile(40, 1580, 1584, "w3")

            # ---------------- software-pipelined chunk stages ----------------
            # PE issue order per iteration i: gathers(i), R(i-1), fin/w2/w3(i-2)
            # so PE never waits on the drain->product chain or the tail chain.
            def lrelu(dst, src, rows):
                if lrelu_mode == "act":
                    nc.scalar.activation(dst[:rows], src[:rows], AF.Lrelu, alpha=SLOPE)
                else:
                    nc.vector.tensor_scalar_mul(dst[:rows], src[:rows], SLOPE)
                    nc.vector.tensor_tensor(dst[:rows], dst[:rows], src[:rows], op=OP.max)

            grp_tiles = {}
            st = {}     # per-chunk live tiles

            def stage_load(c):
                cs = slice(c * nb, (c + 1) * nb)
                xt = xp.tile([ROWS, 2, nb], bf16, tag="xt", bufs=3)
                xm = xp.tile([NO, 2, nb], bf16, tag="xm", bufs=3)
                px8 = xp.tile([8, nb], bf16, tag="px8", bufs=3)
                nc.sync.dma_start(out=xt[:], in_=xD[0:ROWS, :, cs])
                nc.sync.dma_start(out=xm[:], in_=xD[ROWS:ROWS + NO, :, cs])
                nc.scalar.dma_start(out=px8[:], in_=xD[182:190, 0, cs])
                if c % grp == 0:
                    cs4 = slice(c * nb, (c + grp) * nb)
                    o_re2 = sp.tile([2, grp * nb], f32, tag="ore", bufs=max(1, nchunk // grp), name="o_re2")
                    o_im2 = sp.tile([2, grp * nb], f32, tag="oim", bufs=max(1, nchunk // grp), name="o_im2")
                    nc.sync.dma_start(out=o_re2[:], in_=extD[0:2, cs4])
                    nc.sync.dma_start(out=o_im2[:], in_=extD[2:4, cs4])
                    grp_tiles[c // grp] = (o_re2, o_im2)
                st[c] = {"xt": xt, "xm": xm, "px8": px8}

            def stage_gather(c):
                xt = st[c]["xt"]
                pas, pcs = [], []
                for s, (r0, rk) in enumerate(KSPLITS):
                    pa = pp.tile([128, 2, nb], f32, tag="pa", bufs=1)
                    pc = pp.tile([128, 2, nb], f32, tag="pc", bufs=1)
                    a_sl = sel_sb[:, r0:r0 + rk]
                    c_sl = sel_sb[:, NSTACK + r0:NSTACK + r0 + rk]
                    nc.tensor.matmul(pa[:rk, 0, :], a_sl, xt[:, 0, :], start=True, stop=True)
                    nc.tensor.matmul(pa[:rk, 1, :], a_sl, xt[:, 1, :], start=True, stop=True)
                    nc.tensor.matmul(pc[:rk, 0, :], c_sl, xt[:, 0, :], start=True, stop=True)
                    nc.tensor.matmul(pc[:rk, 1, :], c_sl, xt[:, 1, :], start=True, stop=True)
                    pas.append(pa)
                    pcs.append(pc)
                st[c]["pa"] = pas
                st[c]["pc"] = pcs

            def stage_drain(c):
                dr = []
                for s, (r0, rk) in enumerate(KSPLITS):
                    pa, pc = st[c]["pa"][s], st[c]["pc"][s]
                    aS = tp.tile([128, 2, nb], bf16, tag="aS", bufs=2)
                    cS = tp.tile([128, 2, nb], bf16, tag="cS", bufs=2)
                    if s == 0:
                        nc.scalar.copy(aS[:rk], pa[:rk])
                    else:
                        nc.vector.tensor_copy(aS[:rk], pa[:rk])
                    nc.scalar.copy(cS[:rk], pc[:rk])
                    dr.append((aS, cS))
                st[c]["dr"] = dr

            def stage_products(c):
                xt = st[c]["xt"]
                # diagonal power rows: pw = xre^2 + xim^2 (rows 40..81 weighted)
                ud = sp.tile([NDIAG, 2, nb], bf16, tag="ud", bufs=2)
                pw = gp.tile([NDIAG, nb], bf16, tag="pw", bufs=2)
                nc.gpsimd.tensor_tensor(ud[:], xt[:], xt[:], op=OP.mult)
                nc.vector.tensor_tensor(pw[:], ud[:, 0, :], ud[:, 1, :], op=OP.add)
                g_tiles = []
                for s, (r0, rk) in enumerate(KSPLITS):
                    aS, cS = st[c]["dr"][s]
                    m1 = tp.tile([128, nb], bf16, tag="m1", bufs=2)
                    m2 = tp.tile([128, nb], bf16, tag="m2", bufs=2)
                    m3 = tp.tile([128, nb], bf16, tag="m3", bufs=2)
                    m4 = tp.tile([128, nb], bf16, tag="m4", bufs=2)
                    gr = gp.tile([128, nb], bf16, tag=f"gr{s}")
                    gi = gp.tile([128, nb], bf16, tag=f"gi{s}")
                    nc.gpsimd.tensor_tensor(m1[:rk], aS[:rk, 0, :], cS[:rk, 0, :], op=OP.mult)
                    nc.gpsimd.tensor_tensor(m2[:rk], aS[:rk, 1, :], cS[:rk, 1, :], op=OP.mult)
                    nc.gpsimd.tensor_tensor(m3[:rk], aS[:rk, 1, :], cS[:rk, 0, :], op=OP.mult)
                    nc.gpsimd.tensor_tensor(m4[:rk], aS[:rk, 0, :], cS[:rk, 1, :], op=OP.mult)
                    nc.vector.tensor_tensor(gr[:rk], m1[:rk], m2[:rk], op=OP.add)
                    nc.vector.tensor_tensor(gi[:rk], m3[:rk], m4[:rk], op=OP.subtract)
                    g_tiles.append((gr, gi))
                st[c]["g"] = g_tiles
                st[c]["pw"] = pw

            def stage_R(c):
                g_tiles, pw, xm = st[c]["g"], st[c]["pw"], st[c]["xm"]
                p_R = pp.tile([128, 2, nb], f32, tag="pR", bufs=1)
                for s, (r0, rk) in enumerate(KSPLITS):
                    gr, gi = g_tiles[s]
                    rw = rw_sb[s]
                    nc.tensor.matmul(p_R[:NO, 0, :], rw[:rk, 0:NO], gr[:rk],
                                     start=(s == 0), stop=False)
                    nc.tensor.matmul(p_R[:NO, 0, :], rw[:rk, NO:2 * NO], gi[:rk],
                                     start=False, stop=False)
                    nc.tensor.matmul(p_R[:NO, 1, :], rw[:rk, 2 * NO:3 * NO], gr[:rk],
                                     start=(s == 0), stop=False)
                    nc.tensor.matmul(p_R[:NO, 1, :], rw[:rk, 3 * NO:4 * NO], gi[:rk],
                                     start=False, stop=False)
                nc.tensor.matmul(p_R[:NO, 0, :], rwd_sb[:, 0:NO], pw[:], start=False, stop=True)
                nc.tensor.matmul(p_R[:NO, 1, :], rwd_sb[:, NO:2 * NO], pw[:], start=False, stop=True)
                rS = tp.tile([NO, 2, nb], bf16, tag="rS", bufs=2)
                nc.scalar.copy(rS[:], p_R[:NO])
                # T products (xrep * R), contracted by fin with +- weights next iter
                u1 = tp.tile([NO, nb], bf16, tag="u1", bufs=2)
                u2 = tp.tile([NO, nb], bf16, tag="u2", bufs=2)
                u3 = tp.tile([NO, nb], bf16, tag="u3", bufs=2)
                u4 = tp.tile([NO, nb], bf16, tag="u4", bufs=2)
                nc.gpsimd.tensor_tensor(u1[:], xm[:, 0, :], rS[:, 0, :], op=OP.mult)
                nc.gpsimd.tensor_tensor(u2[:], xm[:, 1, :], rS[:, 1, :], op=OP.mult)
                nc.vector.tensor_tensor(u3[:], xm[:, 0, :], rS[:, 1, :], op=OP.mult)
                nc.vector.tensor_tensor(u4[:], xm[:, 1, :], rS[:, 0, :], op=OP.mult)
                st[c]["u"] = (u1, u2, u3, u4)

            def stage_tail(c):
                u1, u2, u3, u4 = st[c]["u"]
                px8 = st[c]["px8"]
                p_h1 = pp.tile([128, nb], f32, tag="ph1", bufs=1)
                nc.tensor.matmul(p_h1[:8], fin_sb[:, 0:8], u1[:], start=True, stop=False)
                nc.tensor.matmul(p_h1[:8], fin_sb[:, 8:16], u2[:], start=False, stop=False)
                nc.tensor.matmul(p_h1[:8], fin_sb[:, 16:24], u3[:], start=False, stop=False)
                nc.tensor.matmul(p_h1[:8], fin_sb[:, 24:32], u4[:], start=False, stop=True)
                h1s = sp.tile([8, nb], bf16, tag="h1s")
                lrelu(h1s, p_h1, 8)
                # P > 0 and lrelu is positively homogeneous: fold P into h1
                h1p = sp.tile([8, nb], bf16, tag="h1p")
                nc.vector.tensor_tensor(h1p[:], h1s[:], px8[:], op=OP.mult)
                p_h2 = pp.tile([128, nb], f32, tag="misc", bufs=1)
                nc.tensor.matmul(p_h2[:40], w2_sb[:], h1p[:8], start=True, stop=True)
                h2s = sp.tile([40, nb], bf16, tag="h2s")
                lrelu(h2s, p_h2, 40)
                p_ere = pp.tile([128, nb], f32, tag="misc", bufs=1)
                p_eim = pp.tile([128, nb], f32, tag="misc", bufs=1)
                nc.tensor.matmul(p_ere[:2], w3_sb[:, 0:2], h2s[:], start=True, stop=True)
                nc.tensor.matmul(p_eim[:2], w3_sb[:, 2:4], h2s[:], start=True, stop=True)
                # out = center + E' (P already folded into E')
                half = (c % grp) * nb
                o_re = grp_tiles[c // grp][0][:, half:half + nb]
                o_im = grp_tiles[c // grp][1][:, half:half + nb]
                nc.vector.tensor_tensor(o_re, p_ere[:2], o_re, op=OP.add)
                nc.vector.tensor_tensor(o_im, p_eim[:2], o_im, op=OP.add)
                if c % grp == grp - 1:
                    g0 = c // grp
                    gsl = slice(g0 * grp * nb, (g0 + 1) * grp * nb)
                    nc.sync.dma_start(out=outD[0:2, gsl], in_=grp_tiles[g0][0][:])
                    nc.sync.dma_start(out=outD[2:4, gsl], in_=grp_tiles[g0][1][:])
                del st[c]

            for i in range(nchunk + 2):
                if i < nchunk:
                    stage_load(i)
                    stage_gather(i)          # PE
                if i >= 1 and i - 1 < nchunk:
                    stage_R(i - 1)           # PE + rS/u
                if i >= 2:
                    stage_tail(i - 2)        # PE fin + MLP tail
                if i < nchunk:
                    stage_drain(i)           # ACT/DVE
                    stage_products(i)        # POOL/DVE
    nc.compile()
    return nc


def _prep_core_inputs(inputs, static, folded):
    """Shard + lay out inputs per core. Returns list of in_maps."""
    import ml_dtypes
    bf16 = ml_dtypes.bfloat16
    xr = np.ascontiguousarray(inputs["x_real"]).reshape(BATCH, ROWS)
    xi = np.ascontiguousarray(inputs["x_imag"]).reshape(BATCH, ROWS)
    t0 = np.ascontiguousarray(inputs["task_info"][:, 0])
    P = (10.0 ** (t0.astype(np.float64) / 10.0)).astype(np.float32)
    consts = pack_consts(static, folded)
    in_maps = []
    for c in range(NCORES):
        s = slice(c * BCORE, (c + 1) * BCORE)
        m = {"consts": consts}
        xrT = np.ascontiguousarray(xr[s].T).astype(bf16)
        xiT = np.ascontiguousarray(xi[s].T).astype(bf16)
        Pb = P[s].astype(bf16)
        xall = np.empty((190, 2, BCORE), bf16)
        xall[0:ROWS, 0, :] = xrT
        xall[0:ROWS, 1, :] = xiT
        xall[ROWS:ROWS + 50, 0, :] = xrT[0:50]
        xall[ROWS + 50:ROWS + 100, 0, :] = xrT[0:50]
        xall[ROWS:ROWS + 50, 1, :] = xiT[0:50]
        xall[ROWS + 50:ROWS + 100, 1, :] = xiT[0:50]
        xall[182:190, 0, :] = Pb[None, :]
        xall[182:190, 1, :] = 0
        m["xall"] = xall
        ext = np.empty((4, BCORE), np.float32)
        ext[0] = xr[s][:, 2 * LH]       # center tap re, pol 0
        ext[1] = xr[s][:, 2 * LH + 1]   # pol 1
        ext[2] = xi[s][:, 2 * LH]
        ext[3] = xi[s][:, 2 * LH + 1]
        m["ext"] = ext
        in_maps.append(m)
    return in_maps


_CACHE = {}


def kernel(**inputs):
    from concourse.bass_utils import run_bass_kernel_spmd

    static = build_static()
    folded = fold_weights(
        np.asarray(inputs["W1_real"]), np.asarray(inputs["W1_imag"]),
        np.asarray(inputs["W2_real"]), np.asarray(inputs["W2_imag"]),
        np.asarray(inputs["W3_real"]), np.asarray(inputs["W3_imag"]),
    )
    if "nc" not in _CACHE:
        _CACHE["nc"] = build_nc()
    nc = _CACHE["nc"]
    in_maps = _prep_core_inputs(inputs, static, folded)
    res = run_bass_kernel_spmd(nc, in_maps, list(range(NCORES)))
    out = np.empty((BATCH, NM, 2), np.float32)
    for c in range(NCORES):
        o4 = res.results[c]["out"]
        s = slice(c * BCORE, (c + 1) * BCORE)
        out[s, 0, 0] = o4[0]
        out[s, 1, 0] = o4[1]
        out[s, 0, 1] = o4[2]
        out[s, 1, 1] = o4[3]
    return out
